# revision 10
# baseline (speedup 1.0000x reference)
"""Causal self-attention (B=4, T=2048, C=1024, 16 heads) on 8 trn2 NeuronCores.

Sharding: tensor-parallel over heads (2 heads/core) for QKV + attention,
then an AllToAll reshards from head-split to token-split for the output
projection.  Each core returns out[token_slice, :]; the host concatenates.

Per-core pipeline (all cores run the identical program; only the fed
W_qkv column-slice differs):
  stage 1: qT,kT  [128ch x 2048tok]  and v [tok-major] per batch, from
           x @ W_qkv_slice  (x is DMA'd in transposed [c, t] tiles)
  stage 2: causal attention per (batch, head): S^T tiles [kt=128, q=512],
           exp on ACT (no max-subtraction: scores/8 ~ N(0,1), bounded),
           multiplicative 0/1 causal mask on diagonal blocks,
           AV accumulation with a ones-column appended to v so PSUM row 64
           carries the softmax denominators; normalize via reciprocal +
           partition_broadcast.
  stage 3: AllToAll (4.2MB/rank) -> y^T [1024ch, 1024tok] token slice,
           out = y^T.T @ W_proj  accumulated over 8 channel chunks.
"""

import os
import numpy as np

from concourse import bass, bacc, mybir, tile
from concourse.bass_utils import run_bass_kernel_spmd

F32 = mybir.dt.float32
F32R = mybir.dt.float32r
BF16 = mybir.dt.bfloat16

B, T, C = 4, 2048, 1024
H, D = 16, 64
NCORES = 8
HPC = H // NCORES            # heads per core = 2
QKC = HPC * D                # per-core q/k/v channels = 128
BT = B * T                   # 8192 tokens total
TPS = BT // NCORES           # tokens per core after A2A = 1024
P = 128
TW = 512                     # token window for stage1/attention q windows
NW = T // TW                 # windows per batch = 4
NKT = T // P                 # kt tiles per batch = 16

# ---- dtype knobs (bitcast matmul operands; float32r = full-rate PE) ----
MM_DT_QKV = F32              # stage-1 matmuls
MM_DT_ATT = F32              # attention matmuls
MM_DT_PROJ = F32             # projection matmuls


def _mm(ap, dt):
    return ap.bitcast(dt) if dt is not F32 else ap


def _causal_mask_01() -> np.ndarray:
    """mask[p, m, f] = 1.0 iff kt_local = 128*m + p <= f, for q windows of 512."""
    m = np.zeros((P, NW, TW), dtype=np.float32)
    p = np.arange(P)[:, None, None]
    mm = np.arange(NW)[None, :, None]
    f = np.arange(TW)[None, None, :]
    m[(P * mm + p) <= f] = 1.0
    return m


def build() -> bass.Bass:
    nc = bacc.Bacc(num_devices=NCORES, target_bir_lowering=False)

    x_d = nc.dram_tensor("x", [BT, C], F32, kind="ExternalInput")
    wqkv_d = nc.dram_tensor("wqkv", [C, 3 * QKC], F32, kind="ExternalInput")
    wproj_d = nc.dram_tensor("wproj", [C, C], F32, kind="ExternalInput")
    out_d = nc.dram_tensor("out", [TPS, C], F32, kind="ExternalOutput")

    mask_d = nc.inline_tensor(_causal_mask_01(), name="mask01")

    KC = C // P  # 8 contraction chunks

    with tile.TileContext(nc) as tc:
        from contextlib import ExitStack

        with ExitStack() as ctx:
            # ---- persistent pools ----
            wq_pool = ctx.enter_context(tc.tile_pool(name="wq", bufs=1))
            msk_pool = ctx.enter_context(tc.tile_pool(name="msk", bufs=1))
            wp_pool = ctx.enter_context(tc.tile_pool(name="wp", bufs=1))
            dram = ctx.enter_context(tc.tile_pool(name="dram", bufs=1, space="DRAM"))

            wqkv_sb = wq_pool.tile([P, KC, 3 * QKC], F32)
            nc.sync.dma_start(
                out=wqkv_sb[:], in_=x_dram_re(wqkv_d, "(k p) n -> p k n")
            )
            mask_sb = msk_pool.tile([P, NW, TW], F32)
            nc.sync.dma_start(out=mask_sb[:], in_=mask_d[:, :, :])
            ones_sb = msk_pool.tile([1, D], F32)
            nc.gpsimd.memset(ones_sb[:], 1.0)
            wproj_sb = wp_pool.tile([P, KC, C], F32)
            nc.sync.dma_start(
                out=wproj_sb[:], in_=x_dram_re(wproj_d, "(k p) n -> p k n")
            )

            y_send = dram.tile([NCORES, QKC, TPS], F32)
            y_recv = dram.tile([NCORES, QKC, TPS], F32)

            # ---- stage 1 + 2 pools ----
            s12 = ExitStack()
            with s12:
                xT_pool = s12.enter_context(tc.tile_pool(name="xT", bufs=2))
                qkv_pool = s12.enter_context(tc.tile_pool(name="qkv", bufs=2))
                ps1 = s12.enter_context(
                    tc.tile_pool(name="ps1", bufs=2, space="PSUM")
                )
                pss = s12.enter_context(
                    tc.tile_pool(name="pss", bufs=3, space="PSUM")
                )
                psy = s12.enter_context(
                    tc.tile_pool(name="psy", bufs=2, space="PSUM")
                )
                pt_pool = s12.enter_context(tc.tile_pool(name="pt", bufs=4))
                nrm_pool = s12.enter_context(tc.tile_pool(name="nrm", bufs=2))
                yt_pool = s12.enter_context(tc.tile_pool(name="yt", bufs=2))

                for b in range(B):
                    qT_b = qkv_pool.tile([P, T], F32, tag="qT")
                    kT_b = qkv_pool.tile([P, T], F32, tag="kT")
                    v_b = qkv_pool.tile([P, NKT, HPC, D + 1], F32, tag="v")
                    # ones column for softmax denominators
                    nc.gpsimd.memset(v_b[:, :, :, D : D + 1], 1.0)

                    # ---- stage 1: qT, kT, v for batch b ----
                    for w in range(NW):
                        t0 = b * T + w * TW
                        xT = xT_pool.tile([P, KC, TW], F32)
                        for kc in range(KC):
                            nc.sync.dma_start(
                                out=xT[:, kc, :],
                                in_=x_d[t0 : t0 + TW, kc * P : (kc + 1) * P].rearrange(
                                    "t p -> p t"
                                ),
                            )
                        for which, dst in ((0, qT_b), (1, kT_b)):
                            ps = ps1.tile([P, TW], F32, tag="ps1")
                            for kc in range(KC):
                                nc.tensor.matmul(
                                    ps[:],
                                    lhsT=_mm(
                                        wqkv_sb[:, kc, which * QKC : (which + 1) * QKC],
                                        MM_DT_QKV,
                                    ),
                                    rhs=_mm(xT[:, kc, :], MM_DT_QKV),
                                    start=(kc == 0),
                                    stop=(kc == KC - 1),
                                )
                            nc.vector.tensor_copy(dst[:, w * TW : (w + 1) * TW], ps[:])
                        for s in range(TW // P):
                            jt = w * (TW // P) + s
                            psv = ps1.tile([P, TW], F32, tag="ps1", name="psv")
                            psv = psv[:, :QKC]
                            for kc in range(KC):
                                nc.tensor.matmul(
                                    psv[:],
                                    lhsT=_mm(xT[:, kc, s * P : (s + 1) * P], MM_DT_QKV),
                                    rhs=_mm(
                                        wqkv_sb[:, kc, 2 * QKC : 3 * QKC], MM_DT_QKV
                                    ),
                                    start=(kc == 0),
                                    stop=(kc == KC - 1),
                                )
                            for h in range(HPC):
                                nc.scalar.copy(
                                    v_b[:, jt, h, 0:D], psv[:, h * D : (h + 1) * D]
                                )

                    # ---- stage 2: attention for batch b ----
                    for h in range(HPC):
                        qT_h = qT_b[h * D : (h + 1) * D, :]
                        kT_h = kT_b[h * D : (h + 1) * D, :]
                        for w in range(NW):
                            nkt = (w + 1) * (TW // P)
                            ps_y = psy.tile([D + 1, TW], F32, tag="ps_y")
                            for jk in range(nkt):
                                ps_s = pss.tile([P, TW], F32, tag="ps_s")
                                nc.tensor.matmul(
                                    ps_s[:],
                                    lhsT=_mm(kT_h[:, jk * P : (jk + 1) * P], MM_DT_ATT),
                                    rhs=_mm(qT_h[:, w * TW : (w + 1) * TW], MM_DT_ATT),
                                    start=True,
                                    stop=True,
                                )
                                pt = pt_pool.tile([P, TW], F32, tag="pt")
                                nc.scalar.activation(
                                    pt[:],
                                    ps_s[:],
                                    mybir.ActivationFunctionType.Exp,
                                    scale=1.0 / np.sqrt(D),
                                )
                                m = jk - w * (TW // P)
                                if m >= 0:
                                    nc.vector.tensor_mul(
                                        pt[:], pt[:], mask_sb[:, m, :]
                                    )
                                nc.tensor.matmul(
                                    ps_y[:],
                                    lhsT=_mm(v_b[:, jk, h, :], MM_DT_ATT),
                                    rhs=_mm(pt[:], MM_DT_ATT),
                                    start=(jk == 0),
                                    stop=(jk == nkt - 1),
                                )
                            recip = nrm_pool.tile([1, TW], F32, tag="recip")
                            nc.vector.reciprocal(recip[:], ps_y[D : D + 1, :])
                            bc_ps = pss.tile([D, TW], F32, tag="ps_s", name="bc_ps")
                            nc.tensor.matmul(
                                bc_ps[:], lhsT=ones_sb[:], rhs=recip[:],
                                start=True, stop=True,
                            )
                            bc = nrm_pool.tile([D, TW], F32, tag="bc")
                            nc.vector.tensor_copy(bc[:], bc_ps[:])
                            yt = yt_pool.tile([D, TW], F32, tag="yt")
                            nc.vector.tensor_mul(yt[:], ps_y[0:D, :], bc[:])
                            g0 = b * T + w * TW
                            shard = g0 // TPS
                            c0 = g0 % TPS
                            nc.sync.dma_start(
                                out=y_send[shard, h * D : (h + 1) * D, c0 : c0 + TW],
                                in_=yt[:],
                            )

            # ---- stage 3: A2A + projection ----
            nc.gpsimd.collective_compute(
                "AllToAll",
                mybir.AluOpType.bypass,
                replica_groups=[list(range(NCORES))],
                ins=[y_send.opt()],
                outs=[y_recv.opt()],
            )

            s3 = ExitStack()
            with s3:
                yr_pool = s3.enter_context(tc.tile_pool(name="yr", bufs=2))
                pso = s3.enter_context(tc.tile_pool(name="pso", bufs=2, space="PSUM"))
                ob_pool = s3.enter_context(tc.tile_pool(name="ob", bufs=2))
                for jt in range(TPS // P):
                    yr = yr_pool.tile([P, KC, P], F32, tag="yr")
                    nc.sync.dma_start(
                        out=yr[:],
                        in_=y_recv[:, :, jt * P : (jt + 1) * P].rearrange(
                            "k p t -> p k t"
                        ),
                    )
                    for half in range(C // TW):
                        ps_o = pso.tile([P, TW], F32, tag="ps_o")
                        for kc in range(KC):
                            nc.tensor.matmul(
                                ps_o[:],
                                lhsT=_mm(yr[:, kc, :], MM_DT_PROJ),
                                rhs=_mm(
                                    wproj_sb[:, kc, half * TW : (half + 1) * TW],
                                    MM_DT_PROJ,
                                ),
                                start=(kc == 0),
                                stop=(kc == KC - 1),
                            )
                        ob = ob_pool.tile([P, TW], F32, tag="ob")
                        nc.vector.tensor_copy(ob[:], ps_o[:])
                        nc.sync.dma_start(
                            out=out_d[jt * P : (jt + 1) * P, half * TW : (half + 1) * TW],
                            in_=ob[:],
                        )

    nc.finalize()
    return nc


def x_dram_re(handle, pattern):
    return handle[:, :].rearrange(pattern, p=P)


_NC_CACHE: dict = {}


def _get_nc() -> bass.Bass:
    if "nc" not in _NC_CACHE:
        _NC_CACHE["nc"] = build()
    return _NC_CACHE["nc"]


def shard_inputs(x, W_qkv, W_proj):
    x = np.ascontiguousarray(np.asarray(x, dtype=np.float32).reshape(BT, C))
    W_qkv = np.asarray(W_qkv, dtype=np.float32)
    W_proj = np.ascontiguousarray(np.asarray(W_proj, dtype=np.float32))
    in_maps = []
    for c in range(NCORES):
        cols = slice(QKC * c, QKC * (c + 1))
        w_c = np.ascontiguousarray(
            np.concatenate(
                [W_qkv[:, cols], W_qkv[:, C:][:, cols], W_qkv[:, 2 * C :][:, cols]],
                axis=1,
            )
        )
        in_maps.append({"x": x, "wqkv": w_c, "wproj": W_proj})
    return in_maps


def run(in_maps, trace=False, **kwargs):
    return run_bass_kernel_spmd(
        _get_nc(), in_maps, core_ids=list(range(NCORES)), trace=trace, **kwargs
    )


def kernel(x, W_qkv, W_proj):
    res = run(shard_inputs(x, W_qkv, W_proj), trace=False)
    out = np.concatenate([res.results[c]["out"] for c in range(NCORES)], axis=0)
    return out.reshape(B, T, C).astype(np.float32)


# revision 16
# speedup vs baseline: 4.6404x; 4.6404x over previous
"""Causal self-attention (B=4, T=2048, C=1024, 16 heads) on 8 trn2 NeuronCores.

Sharding: tensor-parallel over heads (2 heads/core) for QKV + attention,
then an AllToAll reshards from head-split to token-split for the output
projection.  Each core returns out[token_slice, :]; the host concatenates.

Per-core pipeline (all cores run the identical program; only the fed
W_qkv column-slice differs):
  stage 1: qT,kT  [128ch x 2048tok]  and v [tok-major] per batch, from
           x @ W_qkv_slice  (x is DMA'd in transposed [c, t] tiles)
  stage 2: causal attention per (batch, head): S^T tiles [kt=128, q=512],
           exp on ACT (no max-subtraction: scores/8 ~ N(0,1), bounded),
           multiplicative 0/1 causal mask on diagonal blocks,
           AV accumulation with a ones-column appended to v so PSUM row 64
           carries the softmax denominators; normalize via reciprocal +
           partition_broadcast.
  stage 3: AllToAll (4.2MB/rank) -> y^T [1024ch, 1024tok] token slice,
           out = y^T.T @ W_proj  accumulated over 8 channel chunks.
"""

import os
import numpy as np

from concourse import bass, bacc, mybir, tile
from concourse.bass_utils import run_bass_kernel_spmd

F32 = mybir.dt.float32
F32R = mybir.dt.float32r
BF16 = mybir.dt.bfloat16

B, T, C = 4, 2048, 1024
H, D = 16, 64
NCORES = 8
HPC = H // NCORES            # heads per core = 2
QKC = HPC * D                # per-core q/k/v channels = 128
BT = B * T                   # 8192 tokens total
TPS = BT // NCORES           # tokens per core after A2A = 1024
P = 128
TW = 512                     # token window for stage1/attention q windows
NW = T // TW                 # windows per batch = 4
NKT = T // P                 # kt tiles per batch = 16

# ---- dtype knobs (bitcast matmul operands; float32r = full-rate PE) ----
_DT = {"f32": F32, "f32r": F32R}[os.environ.get("KMM_DT", "f32")]
MM_DT_QKV = _DT              # stage-1 matmuls
MM_DT_ATT = _DT              # attention matmuls
MM_DT_PROJ = _DT             # projection matmuls


def _mm(ap, dt):
    return ap.bitcast(dt) if dt is not F32 else ap


def _causal_mask_01() -> np.ndarray:
    """mask[p, m, f] = 1.0 iff kt_local = 128*m + p <= f, for q windows of 512."""
    m = np.zeros((P, NW, TW), dtype=np.float32)
    p = np.arange(P)[:, None, None]
    mm = np.arange(NW)[None, :, None]
    f = np.arange(TW)[None, None, :]
    m[(P * mm + p) <= f] = 1.0
    return m


def build() -> bass.Bass:
    nc = bacc.Bacc(num_devices=NCORES, target_bir_lowering=False)

    x_d = nc.dram_tensor("x", [BT, C], F32, kind="ExternalInput")
    wqkv_d = nc.dram_tensor("wqkv", [C, 3 * QKC], F32, kind="ExternalInput")
    wproj_d = nc.dram_tensor("wproj", [C, C], F32, kind="ExternalInput")
    out_d = nc.dram_tensor("out", [TPS, C], F32, kind="ExternalOutput")

    mask_d = nc.inline_tensor(_causal_mask_01(), name="mask01")
    ident_d = nc.inline_tensor(np.eye(P, dtype=np.float32), name="ident")

    KC = C // P  # 8 contraction chunks

    with tile.TileContext(nc) as tc:
        from contextlib import ExitStack

        with ExitStack() as ctx:
            # ---- persistent pools ----
            wq_pool = ctx.enter_context(tc.tile_pool(name="wq", bufs=1))
            msk_pool = ctx.enter_context(tc.tile_pool(name="msk", bufs=1))
            wp_pool = ctx.enter_context(tc.tile_pool(name="wp", bufs=1))
            dram = ctx.enter_context(tc.tile_pool(name="dram", bufs=1, space="DRAM"))

            wqkv_sb = wq_pool.tile([P, KC, 3 * QKC], F32)
            nc.sync.dma_start(
                out=wqkv_sb[:], in_=x_dram_re(wqkv_d, "(k p) n -> p k n")
            )
            mask_sb = msk_pool.tile([P, NW, TW], F32)
            nc.sync.dma_start(out=mask_sb[:], in_=mask_d[:, :, :])
            ones_sb = msk_pool.tile([1, D], F32)
            nc.gpsimd.memset(ones_sb[:], 1.0)
            ident_sb = msk_pool.tile([P, P], F32)
            nc.sync.dma_start(out=ident_sb[:], in_=ident_d[:, :])
            wproj_sb = wp_pool.tile([P, KC, C], F32)
            nc.sync.dma_start(
                out=wproj_sb[:], in_=x_dram_re(wproj_d, "(k p) n -> p k n")
            )

            y_send = dram.tile([NCORES, QKC, TPS], F32)
            y_recv = dram.tile([NCORES, QKC, TPS], F32)

            # ---- stage 1 + 2 pools ----
            s12 = ExitStack()
            with s12:
                xT_pool = s12.enter_context(tc.tile_pool(name="xT", bufs=2))
                qkv_pool = s12.enter_context(tc.tile_pool(name="qkv", bufs=2))
                ps1 = s12.enter_context(
                    tc.tile_pool(name="ps1", bufs=2, space="PSUM")
                )
                pss = s12.enter_context(
                    tc.tile_pool(name="pss", bufs=3, space="PSUM")
                )
                psy = s12.enter_context(
                    tc.tile_pool(name="psy", bufs=2, space="PSUM")
                )
                pt_pool = s12.enter_context(tc.tile_pool(name="pt", bufs=4))
                nrm_pool = s12.enter_context(tc.tile_pool(name="nrm", bufs=2))
                yt_pool = s12.enter_context(tc.tile_pool(name="yt", bufs=2))

                for b in range(B):
                    qT_b = qkv_pool.tile([P, T], F32, tag="qT")
                    kT_b = qkv_pool.tile([P, T], F32, tag="kT")
                    v_b = qkv_pool.tile([P, NKT, HPC, D + 1], F32, tag="v")
                    # ones column for softmax denominators
                    nc.gpsimd.memset(v_b[:, :, :, D : D + 1], 1.0)

                    # ---- stage 1: qT, kT, v for batch b ----
                    for w in range(NW):
                        t0 = b * T + w * TW
                        # natural-layout x subtiles (contiguous 4KB-row DMAs)
                        xns = []
                        for s in range(TW // P):
                            xn = xT_pool.tile([P, C], F32, tag="xn", name="xn", bufs=6)
                            nc.sync.dma_start(
                                out=xn[:], in_=x_d[t0 + s * P : t0 + (s + 1) * P, :]
                            )
                            xns.append(xn)
                        # transpose to xT [c-part, tok] on the PE
                        xT = xT_pool.tile([P, KC, TW], F32)
                        for kc in range(KC):
                            ps_t = ps1.tile([P, TW], F32, tag="ps1", name="ps_t")
                            for s in range(TW // P):
                                nc.tensor.transpose(
                                    ps_t[:, s * P : (s + 1) * P],
                                    xns[s][:, kc * P : (kc + 1) * P],
                                    ident_sb[:],
                                )
                            nc.vector.tensor_copy(xT[:, kc, :], ps_t[:])
                        for which, dst in ((0, qT_b), (1, kT_b)):
                            ps = ps1.tile([P, TW], F32, tag="ps1")
                            for kc in range(KC):
                                nc.tensor.matmul(
                                    ps[:],
                                    lhsT=_mm(
                                        wqkv_sb[:, kc, which * QKC : (which + 1) * QKC],
                                        MM_DT_QKV,
                                    ),
                                    rhs=_mm(xT[:, kc, :], MM_DT_QKV),
                                    start=(kc == 0),
                                    stop=(kc == KC - 1),
                                )
                            nc.vector.tensor_copy(dst[:, w * TW : (w + 1) * TW], ps[:])
                        for s in range(TW // P):
                            jt = w * (TW // P) + s
                            psv = ps1.tile([P, TW], F32, tag="ps1", name="psv")
                            psv = psv[:, :QKC]
                            for kc in range(KC):
                                nc.tensor.matmul(
                                    psv[:],
                                    lhsT=_mm(xT[:, kc, s * P : (s + 1) * P], MM_DT_QKV),
                                    rhs=_mm(
                                        wqkv_sb[:, kc, 2 * QKC : 3 * QKC], MM_DT_QKV
                                    ),
                                    start=(kc == 0),
                                    stop=(kc == KC - 1),
                                )
                            for h in range(HPC):
                                nc.scalar.copy(
                                    v_b[:, jt, h, 0:D], psv[:, h * D : (h + 1) * D]
                                )

                    # ---- stage 2: attention for batch b ----
                    for h in range(HPC):
                        qT_h = qT_b[h * D : (h + 1) * D, :]
                        kT_h = kT_b[h * D : (h + 1) * D, :]
                        for w in range(NW):
                            nkt = (w + 1) * (TW // P)
                            ps_y = psy.tile([D + 1, TW], F32, tag="ps_y")
                            for jk in range(nkt):
                                ps_s = pss.tile([P, TW], F32, tag="ps_s")
                                nc.tensor.matmul(
                                    ps_s[:],
                                    lhsT=_mm(kT_h[:, jk * P : (jk + 1) * P], MM_DT_ATT),
                                    rhs=_mm(qT_h[:, w * TW : (w + 1) * TW], MM_DT_ATT),
                                    start=True,
                                    stop=True,
                                )
                                pt = pt_pool.tile([P, TW], F32, tag="pt")
                                nc.scalar.activation(
                                    pt[:],
                                    ps_s[:],
                                    mybir.ActivationFunctionType.Exp,
                                    scale=1.0 / np.sqrt(D),
                                )
                                m = jk - w * (TW // P)
                                if m >= 0:
                                    nc.gpsimd.tensor_mul(
                                        pt[:], pt[:], mask_sb[:, m, :]
                                    )
                                nc.tensor.matmul(
                                    ps_y[:],
                                    lhsT=_mm(v_b[:, jk, h, :], MM_DT_ATT),
                                    rhs=_mm(pt[:], MM_DT_ATT),
                                    start=(jk == 0),
                                    stop=(jk == nkt - 1),
                                )
                            recip = nrm_pool.tile([1, TW], F32, tag="recip")
                            nc.vector.reciprocal(recip[:], ps_y[D : D + 1, :])
                            bc_ps = pss.tile([D, TW], F32, tag="ps_s", name="bc_ps")
                            nc.tensor.matmul(
                                bc_ps[:], lhsT=ones_sb[:], rhs=recip[:],
                                start=True, stop=True,
                            )
                            bc = nrm_pool.tile([D, TW], F32, tag="bc")
                            nc.vector.tensor_copy(bc[:], bc_ps[:])
                            yt = yt_pool.tile([D, TW], F32, tag="yt")
                            nc.vector.tensor_mul(yt[:], ps_y[0:D, :], bc[:])
                            g0 = b * T + w * TW
                            shard = g0 // TPS
                            c0 = g0 % TPS
                            nc.sync.dma_start(
                                out=y_send[shard, h * D : (h + 1) * D, c0 : c0 + TW],
                                in_=yt[:],
                            )

            # ---- stage 3: A2A + projection ----
            nc.gpsimd.collective_compute(
                "AllToAll",
                mybir.AluOpType.bypass,
                replica_groups=[list(range(NCORES))],
                ins=[y_send.opt()],
                outs=[y_recv.opt()],
            )

            s3 = ExitStack()
            with s3:
                yr_pool = s3.enter_context(tc.tile_pool(name="yr", bufs=2))
                pso = s3.enter_context(tc.tile_pool(name="pso", bufs=2, space="PSUM"))
                ob_pool = s3.enter_context(tc.tile_pool(name="ob", bufs=2))
                for jt in range(TPS // P):
                    yr = yr_pool.tile([P, KC, P], F32, tag="yr")
                    nc.sync.dma_start(
                        out=yr[:],
                        in_=y_recv[:, :, jt * P : (jt + 1) * P].rearrange(
                            "k p t -> p k t"
                        ),
                    )
                    for half in range(C // TW):
                        ps_o = pso.tile([P, TW], F32, tag="ps_o")
                        for kc in range(KC):
                            nc.tensor.matmul(
                                ps_o[:],
                                lhsT=_mm(yr[:, kc, :], MM_DT_PROJ),
                                rhs=_mm(
                                    wproj_sb[:, kc, half * TW : (half + 1) * TW],
                                    MM_DT_PROJ,
                                ),
                                start=(kc == 0),
                                stop=(kc == KC - 1),
                            )
                        ob = ob_pool.tile([P, TW], F32, tag="ob")
                        nc.vector.tensor_copy(ob[:], ps_o[:])
                        nc.sync.dma_start(
                            out=out_d[jt * P : (jt + 1) * P, half * TW : (half + 1) * TW],
                            in_=ob[:],
                        )

    nc.finalize()
    return nc


def x_dram_re(handle, pattern):
    return handle[:, :].rearrange(pattern, p=P)


_NC_CACHE: dict = {}


def _get_nc() -> bass.Bass:
    if "nc" not in _NC_CACHE:
        _NC_CACHE["nc"] = build()
    return _NC_CACHE["nc"]


def shard_inputs(x, W_qkv, W_proj):
    x = np.ascontiguousarray(np.asarray(x, dtype=np.float32).reshape(BT, C))
    W_qkv = np.asarray(W_qkv, dtype=np.float32)
    W_proj = np.ascontiguousarray(np.asarray(W_proj, dtype=np.float32))
    in_maps = []
    for c in range(NCORES):
        cols = slice(QKC * c, QKC * (c + 1))
        w_c = np.ascontiguousarray(
            np.concatenate(
                [W_qkv[:, cols], W_qkv[:, C:][:, cols], W_qkv[:, 2 * C :][:, cols]],
                axis=1,
            )
        )
        in_maps.append({"x": x, "wqkv": w_c, "wproj": W_proj})
    return in_maps


def run(in_maps, trace=False, **kwargs):
    return run_bass_kernel_spmd(
        _get_nc(), in_maps, core_ids=list(range(NCORES)), trace=trace, **kwargs
    )


def kernel(x, W_qkv, W_proj):
    res = run(shard_inputs(x, W_qkv, W_proj), trace=False)
    out = np.concatenate([res.results[c]["out"] for c in range(NCORES)], axis=0)
    return out.reshape(B, T, C).astype(np.float32)
